# revision 23
# baseline (speedup 1.0000x reference)
"""Trainium2 Bass kernel for nn_Attention_60000693125929.

RMSNorm -> fused QKV proj -> interleaved RoPE -> causal attention -> out proj.
x: [4, 2048, 2048] f32.  8 NeuronCores: shard batch (4) x head-group (2x8 heads).

Per-core dataflow (bf16 matmul inputs, fp32 PSUM accumulation):
  1. Pass streams x tiles: ACT casts to bf16 and computes row sum-of-squares
     (Square + accum_out); the x-tile transpose into xsT quarters is done by
     the XBAR DMA-transpose engine (dma_start_transpose) instead of PE, so
     phase-1 PE does only V = xs @ Wv (spilled to DRAM).  rinv = 1/sqrt(mean
     +eps) is folded into cos/sin tables (built once at phase end) and into
     the V normalize.
  2. qT/kT = W^T-stationary matmuls ([e, n] layout, dh on partitions);
     interleaved RoPE: rotate_half is a partition pair-swap done by two
     strided SBUF->SBUF DMAs (sign of the rotation folded into the sin
     table host-side), then cos/sin elementwise on DVE.  No PE rotation
     matmul.
  3. V is reloaded from DRAM ONCE into 8 SBUF-resident strips at attention
     start (vs per-i-chunk reloads).  Per (i-chunk, head): S^T[j,i] blocks =
     kT.T @ qT; causal handled by skipping j>i blocks and computing only the
     valid [lo:512] column range on diagonal blocks; the in-block triangle
     mask is a GPSIMD elementwise multiply with a 0/1 triangle (no PE mask
     matmul).  exp on ACT, AV + rowsum accumulate, fast reciprocal
     normalize.  The PREVIOUS i-chunk's out projection (outT.T @ WoutT) is
     interleaved into the head loop at a fixed rate so the PE always has
     ACT-independent work; y is written out in [128,512] pieces on rotating
     DMA queues (4 queues for the tail chunk).
"""
import numpy as np
import ml_dtypes
from contextlib import ExitStack

import concourse.bass as bass
import concourse.tile as tile
from concourse import bacc, mybir
from concourse.bass_utils import run_bass_kernel_spmd

F32 = mybir.dt.float32
BF16 = mybir.dt.bfloat16
AF = mybir.ActivationFunctionType
OP = mybir.AluOpType

B, N, D, H, DH = 4, 2048, 2048, 16, 128
HPC = 8                 # heads per core
EQK = 2 * HPC * DH      # 2048 q+k columns per core
EV = HPC * DH           # 1024 v columns per core
EPS = 1.1920929e-07
SCALE = DH ** -0.5
NT = N // 128           # 16 n-tiles
DT = D // 128           # 16 d-tiles
NCH = N // 512          # 4 n-chunks

_NC_CACHE = {}


def build_nc():
    if "nc" in _NC_CACHE:
        return _NC_CACHE["nc"]
    nc = bacc.Bacc("TRN2", target_bir_lowering=False, debug=False)

    # weights arrive host-repacked so every DMA is contiguous per
    # partition (the rings are descriptor-rate-bound: 256B-run rearrange
    # loads cost ~16x more ring time than 4-8KB contiguous runs)
    x = nc.dram_tensor("x", [N, D], F32, kind="ExternalInput").ap()
    wqk = nc.dram_tensor("wqk", [128, 16, DT, 128], BF16,
                         kind="ExternalInput").ap()
    wv = nc.dram_tensor("wv", [128, 2, DT, 512], BF16,
                        kind="ExternalInput").ap()
    wout = nc.dram_tensor("wout", [128, HPC, D], BF16,
                          kind="ExternalInput").ap()
    cos_d = nc.dram_tensor("cos_t", [DH, N], BF16, kind="ExternalInput").ap()
    sin_d = nc.dram_tensor("sin_t", [DH, N], BF16, kind="ExternalInput").ap()
    id_d = nc.dram_tensor("ident", [128, 128], BF16, kind="ExternalInput").ap()
    on_d = nc.dram_tensor("onesm", [128, 128], BF16, kind="ExternalInput").ap()
    tri_d = nc.dram_tensor("tri01", [128, 128], BF16,
                           kind="ExternalInput").ap()
    y = nc.dram_tensor("y", [N, D], F32, kind="ExternalOutput").ap()

    with tile.TileContext(nc) as tc, ExitStack() as ctx:
        const_p = ctx.enter_context(tc.tile_pool(name="const", bufs=1))
        small_p = ctx.enter_context(tc.tile_pool(name="small", bufs=1))
        psum = ctx.enter_context(tc.tile_pool(name="psum", bufs=4, space="PSUM"))
        po_p = ctx.enter_context(tc.tile_pool(name="pop", bufs=2, space="PSUM"))
        misc_p = ctx.enter_context(tc.tile_pool(name="miscp", bufs=2,
                                                space="PSUM"))

        id_s = const_p.tile([128, 128], BF16, tag="ident")
        on_s = const_p.tile([128, 128], BF16, tag="ones")
        tri_s = const_p.tile([128, 128], BF16, tag="tri")

        ssqA = small_p.tile([128, NT], F32, tag="ssqA")
        ssqB = small_p.tile([128, NT], F32, tag="ssqB")
        rms = small_p.tile([128, NT], F32, tag="rms")
        rinv = small_p.tile([128, NT], F32, tag="rinv")
        rinvb = small_p.tile([128, NT], BF16, tag="rinvb")
        eps_s = small_p.tile([128, 1], F32, tag="eps")
        nc.vector.memzero(eps_s[:])
        nc.vector.tensor_scalar_add(eps_s[:], eps_s[:], EPS)

        # long-lived: qkT e-tiles (q: 0..7, k: 8..15)
        qk_p = ctx.enter_context(tc.tile_pool(name="qk", bufs=1))
        qkT = [qk_p.tile([128, N], BF16, tag=f"qkT{et}", name=f"qkT{et}")
               for et in range(16)]

        # V lives SBUF-resident for the whole kernel: phase 1 writes the
        # normalized V projection straight into these strips (DVE writes
        # have fine-grained dependencies), attention reads them.  No DRAM
        # spill/reload.  [128, 4, 512] strips keyed (i-chunk-group, e-half).
        vr_p = ctx.enter_context(tc.tile_pool(name="vres", bufs=1))
        vres = [[vr_p.tile([128, 4, 512], BF16, tag=f"vres{icg}{g}",
                           name=f"vres{icg}{g}")
                 for g in range(2)] for icg in range(NCH)]

        def emit_v(tv, ech, wv_t):
            qv, tqv = tv // 4, tv % 4
            pv = psum.tile([128, 512], F32, tag="mm2", name="pv")
            for dt_i in range(DT):
                nc.tensor.matmul(
                    pv[:],
                    xsT[qv][:, dt_i, tqv * 128:(tqv + 1) * 128],
                    wv_t[:, dt_i, :],
                    start=(dt_i == 0), stop=(dt_i == DT - 1))
            # normalized V goes straight into its resident strip
            nc.vector.tensor_scalar_mul(
                vres[qv][ech][:, tqv, :], pv[:], rinv[:, tv:tv + 1])

        # ---- phase 1: RMSNorm + DMA-transpose + V (ech0) ---------------
        es1 = ExitStack()
        if True:
            xsT_p = es1.enter_context(tc.tile_pool(name="xsTp", bufs=1))
            xsT = [xsT_p.tile([128, DT, 512], BF16, tag=f"xsT{q}",
                               name=f"xsT{q}")
                   for q in range(4)]
            with tc.tile_pool(name="ph1", bufs=4) as ph1_p, \
                 tc.tile_pool(name="ph1b", bufs=2) as ph1b_p, \
                 tc.tile_pool(name="wvp", bufs=1) as wv_p:
                # only the ech0 half of wv lives in phase 1; the ech1
                # V-projection is interleaved into phase 2 (wv_s1 loads at
                # phase-1 end), halving phase-1 wv SBUF
                wv_s0 = wv_p.tile([128, DT, 512], BF16, tag="wv0")
                # startup/ring plan: gpsimd streams [id, wv_s0, on,
                # tri, wv_s1, cos, sin] then the late-odd x tiles; scalar
                # streams the early x tiles (halves) then the even ones;
                # sync is dedicated to the XBAR transposes of chunks 1-3.
                # Chunk 0 is transposed on the PE (it is idle anyway until
                # wv arrives), which also gives the first V matmuls their
                # input without waiting on the slow first DMA-transpose.
                nc.gpsimd.dma_start(id_s[:], id_d)
                nc.gpsimd.dma_start(wv_s0[:], wv[:, 0])
                nc.gpsimd.dma_start(on_s[:], on_d)
                nc.gpsimd.dma_start(tri_s[:], tri_d)

                for t in range(NT):
                    q, tq = t // 4, t % 4
                    xa = ph1_p.tile([128, 1024], F32, tag="xin", name="xa")
                    xb = ph1_p.tile([128, 1024], F32, tag="xin", name="xb")
                    ring = nc.scalar if (t <= 4 or t % 2 == 0) else nc.gpsimd
                    ring.dma_start(xa[:], x[t * 128:(t + 1) * 128, 0:1024])
                    ring.dma_start(xb[:], x[t * 128:(t + 1) * 128, 1024:])
                    xr = ph1b_p.tile([128, D], BF16, tag="xraw")
                    nc.scalar.activation(xr[:, 0:512], xa[:, 0:512], AF.Copy)
                    nc.vector.tensor_copy(xr[:, 512:1024], xa[:, 512:1024])
                    nc.scalar.activation(xr[:, 1024:1536], xb[:, 0:512],
                                         AF.Copy)
                    nc.vector.tensor_copy(xr[:, 1536:2048], xb[:, 512:1024])
                    # row sum-of-squares on ACT, in place on the halves
                    # (the casts above already consumed them)
                    nc.scalar.activation(xa[:], xa[:], AF.Square,
                                         accum_out=ssqA[:, t:t + 1])
                    nc.scalar.activation(xb[:], xb[:], AF.Square,
                                         accum_out=ssqB[:, t:t + 1])
                    nc.vector.tensor_tensor(ssqA[:, t:t + 1],
                                            ssqA[:, t:t + 1],
                                            ssqB[:, t:t + 1], OP.add)
                    nc.scalar.activation(rms[:, t:t + 1], ssqA[:, t:t + 1],
                                         AF.Sqrt, bias=eps_s[:],
                                         scale=1.0 / D)
                    nc.vector.reciprocal(rinv[:, t:t + 1], rms[:, t:t + 1])
                    if q == 0:
                        # PE-path transpose for chunk 0
                        for q4 in range(4):
                            pt = psum.tile([128, 4, 128], BF16, tag="mm2")
                            for j in range(4):
                                dt_i = 4 * q4 + j
                                nc.tensor.transpose(
                                    pt[:, j, :],
                                    xr[:, dt_i * 128:(dt_i + 1) * 128],
                                    id_s[:])
                            nc.vector.tensor_copy(
                                xsT[0][:, 4 * q4:4 * q4 + 4,
                                       tq * 128:(tq + 1) * 128],
                                pt[:])
                    if t > 0:
                        emit_v(t - 1, 0, wv_s0)
                    if q > 0:
                        nc.sync.dma_start_transpose(
                            xsT[q][:, :, tq * 128:(tq + 1) * 128], xr[:])
                emit_v(NT - 1, 0, wv_s0)

            # phase-1 pools just closed; cos/sin tables + the second wv
            # half load into the freed space on gpsimd while the PE runs
            # the last V/qkT boundary work
            ropec_p = es1.enter_context(tc.tile_pool(name="ropec", bufs=1))
            wv1_p = es1.enter_context(tc.tile_pool(name="wv1p", bufs=1))
            cos_s = ropec_p.tile([DH, N], BF16, tag="cos")
            sin_s = ropec_p.tile([DH, N], BF16, tag="sin")
            wv_s1 = wv1_p.tile([128, DT, 512], BF16, tag="wv1")
            nc.gpsimd.dma_start(wv_s1[:], wv[:, 1])
            nc.gpsimd.dma_start(cos_s[:], cos_d)
            nc.gpsimd.dma_start(sin_s[:], sin_d)
            # rinv -> row form -> cos/sin tables pre-scaled by rinv[n]
            nc.vector.tensor_copy(rinvb[:], rinv[:])
            for c in range(NCH):
                prow = misc_p.tile([1, 512], F32, tag="av")
                for tq in range(4):
                    t = 4 * c + tq
                    nc.tensor.matmul(
                        prow[:, tq * 128:(tq + 1) * 128],
                        rinvb[:, t:t + 1], id_s[:],
                        start=True, stop=True)
                rrow = ropec_p.tile([1, 512], BF16, tag="rrow")
                nc.vector.tensor_copy(rrow[:], prow[:])
                pb = misc_p.tile([128, 512], F32, tag="av")
                nc.tensor.matmul(pb[:], on_s[0:1, :], rrow[:],
                                 start=True, stop=True)
                sl = slice(c * 512, (c + 1) * 512)
                # in-place rinv-scaling (saves two [DH, N] tiles)
                nc.vector.tensor_tensor(cos_s[:, sl], cos_s[:, sl],
                                        pb[:], OP.mult)
                nc.vector.tensor_tensor(sin_s[:, sl], sin_s[:, sl],
                                        pb[:], OP.mult)

            # ---- phase 2: qkT + RoPE (inside xsT scope) -----------------
            # rotate_half is a partition pair-swap: two strided SBUF->SBUF
            # DMAs on gpsimd (SWDGE); the rotation signs live in the sin
            # table (host-prepped).  No PE rotation matmul.
            with tc.tile_pool(name="wqkp", bufs=2) as wqk_p, \
                 tc.tile_pool(name="ropeA", bufs=3) as ropeA_p, \
                 tc.tile_pool(name="ropeB", bufs=2) as ropeB_p:
                order = [v for pair in zip(range(8), range(8, 16))
                         for v in pair]
                def emit_rope(st):
                    et_, n0_, raw_, rsw_ = st
                    t1 = ropeB_p.tile([128, 512], BF16, tag="t1")
                    nc.vector.tensor_tensor(
                        t1[:], raw_[:], cos_s[:, n0_:n0_ + 512], OP.mult)
                    t2 = ropeB_p.tile([128, 512], BF16, tag="t2")
                    nc.vector.tensor_tensor(
                        t2[:], rsw_[:], sin_s[:, n0_:n0_ + 512], OP.mult)
                    nc.vector.tensor_add(
                        qkT[et_][:, n0_:n0_ + 512], t1[:], t2[:])

                pending = []
                for gi, et in enumerate(order):
                    wt = wqk_p.tile([128, DT, 128], BF16, tag="wqk")
                    nc.gpsimd.dma_start(wt[:], wqk[:, et])
                    for nch in range(NCH):
                        n0 = nch * 512
                        pq = psum.tile([128, 512], F32, tag="mm2")
                        for dt_i in range(DT):
                            nc.tensor.matmul(
                                pq[:], wt[:, dt_i, :],
                                xsT[nch][:, dt_i, :],
                                start=(dt_i == 0), stop=(dt_i == DT - 1))
                        raw = ropeA_p.tile([128, 512], BF16, tag="raw")
                        nc.scalar.activation(raw[:], pq[:], AF.Copy,
                                             bias=0.0, scale=1.0)
                        rsw = ropeA_p.tile([128, 512], BF16, tag="rsw")
                        nc.gpsimd.dma_start(rsw[0:128:2, :],
                                            raw[1:128:2, :])
                        nc.gpsimd.dma_start(rsw[1:128:2, :],
                                            raw[0:128:2, :])
                        # lag the rope DVE work two steps so neither the
                        # ACT copy nor the swap-DMA latency ever stalls the
                        # PE's in-order queue
                        pending.append((et, n0, raw, rsw))
                        if len(pending) > 2:
                            emit_rope(pending.pop(0))
                    # ech1 V-projection rides along (wv_s1 streamed in at
                    # phase-1 end), lagged so its weights have landed
                    if gi >= 5:
                        emit_v(gi - 5, 1, wv_s1)
                for tv in range(NT - 5, NT):
                    emit_v(tv, 1, wv_s1)
                while pending:
                    emit_rope(pending.pop(0))

        es1.close()  # frees xsT/ropec SBUF before phase-3 pools open

        # ---- phase 3+4: causal attention + out projection ---------------
        with tc.tile_pool(name="outp", bufs=1) as out_p, \
             tc.tile_pool(name="exps", bufs=8) as exps_p, \
             tc.tile_pool(name="att", bufs=3) as att_p, \
             tc.tile_pool(name="woutp", bufs=1) as wo_p, \
             tc.tile_pool(name="ybufp", bufs=3) as y_p:
            # wo in two et-half tiles (whole-tile DMA dependency: the
            # first interleaved out-proj matmuls at ic1 must not wait for
            # all 4MB), loaded AFTER the V strips are queued
            wo_a = wo_p.tile([128, 4, D], BF16, tag="woa")
            wo_b = wo_p.tile([128, 4, D], BF16, tag="wob")
            nc.scalar.dma_start(wo_a[:], wout[:, 0:4])
            nc.scalar.dma_start(wo_b[:], wout[:, 4:8])
            outT = [out_p.tile([128, HPC, 512], BF16, tag=f"outT{q}",
                                name=f"outT{q}")
                    for q in range(4)]
            deferred = [None]

            def finalize_head(st):
                ic_, h_, po_, racc_ = st
                pr = misc_p.tile([128, 512], F32, tag="av")
                nc.tensor.matmul(pr[:], on_s[:], racc_[:],
                                 start=True, stop=True)
                rec = att_p.tile([128, 512], F32, tag="rec")
                rsc = att_p.tile([128, 512], F32, tag="rsc")
                nc.vector.reciprocal_approx_accurate(rec[:], pr[:], rsc[:])
                nc.vector.tensor_tensor(
                    outT[ic_][:, h_, :], po_[:], rec[:], OP.mult)

            # interleaved out-projection of the previous i-chunk
            ops = dict(pieces=[], carry=0.0, rate=0.0, py=None, et=0, dq=0,
                       dmaq=[nc.sync])

            def op_begin(pic, steps, dmaq=None):
                ops["pieces"] = [(4 * pic + tq, dch)
                                 for tq in range(4) for dch in range(4)]
                ops["rate"] = (16.0 * HPC) / steps
                ops["carry"] = 0.0
                ops["py"] = None
                ops["pic"] = pic
                if dmaq is not None:
                    ops["dmaq"] = dmaq

            def op_step(force=False):
                if force:
                    n = 1 << 30
                else:
                    ops["carry"] += ops["rate"]
                    n = int(ops["carry"])
                    ops["carry"] -= n
                while n > 0 and ops["pieces"]:
                    t, dch = ops["pieces"][0]
                    if ops["py"] is None:
                        ops["py"] = misc_p.tile([128, 512], F32, tag="av",
                                                name="oppy")
                        ops["et"] = 0
                    et = ops["et"]
                    tq = t % 4
                    wo_t = wo_a if et < 4 else wo_b
                    nc.tensor.matmul(
                        ops["py"][:],
                        outT[ops["pic"]][:, et, tq * 128:(tq + 1) * 128],
                        wo_t[:, et % 4, dch * 512:(dch + 1) * 512],
                        start=(et == 0), stop=(et == HPC - 1))
                    ops["et"] += 1
                    n -= 1
                    if ops["et"] == HPC:
                        yb = y_p.tile([128, 512], F32, tag="yb")
                        nc.vector.tensor_copy(yb[:], ops["py"][:])
                        qd = ops["dmaq"][ops["dq"] % len(ops["dmaq"])]
                        ops["dq"] += 1
                        qd.dma_start(
                            y[t * 128:(t + 1) * 128,
                              dch * 512:(dch + 1) * 512], yb[:])
                        ops["pieces"].pop(0)
                        ops["py"] = None

            for ic in range(NCH):
                i0 = ic * 512
                njt = 4 * ic + 4
                if ic >= 1:
                    op_begin(ic - 1, 8 * (njt + 4))
                prev = None   # leftover AV drains of the previous head

                def drain_one(st):
                    jt_, lo_, es_ = st["pend"].pop(0)
                    nc.tensor.matmul(
                        st["po"][:, lo_:512],
                        vres[jt_ // 4][st["g"]][:, jt_ % 4,
                                                st["hc"]:st["hc"] + 128],
                        es_[:, lo_:512],
                        start=(jt_ == 0), stop=(jt_ == st["njt"] - 1))

                for h in range(HPC):
                    g = h // 4
                    hc = (h % 4) * 128
                    po = po_p.tile([128, 512], F32, tag="av")
                    # QK+exp run ~4 tiles ahead of AV so the PE's in-order
                    # queue never waits on the ACT exp.  Row sums accumulate
                    # on DVE (racc) -> a single ones-matmul per (ic, h).
                    # The last drains of the previous head carry over into
                    # this head's QK stream so they never stall on the
                    # previous head's tail exps.
                    racc = att_p.tile([128, 512], BF16, tag="racc")
                    cur = dict(pend=[], po=po, g=g, hc=hc, njt=njt)
                    pend = cur["pend"]
                    fin_done = False

                    for jt in range(njt):
                        r = jt - 4 * ic
                        lo = max(0, 128 * r)
                        psq = psum.tile([128, 512], F32, tag="mm2")
                        nc.tensor.matmul(
                            psq[:, lo:512],
                            qkT[HPC + h][:, jt * 128:(jt + 1) * 128],
                            qkT[h][:, i0 + lo:i0 + 512],
                            start=True, stop=True)
                        es = exps_p.tile([128, 512], BF16, tag="es")
                        nc.scalar.activation(es[:, lo:], psq[:, lo:512],
                                             AF.Exp, bias=0.0, scale=SCALE)
                        if r >= 0:
                            # in-block causal mask: zero the strict lower
                            # triangle of the diagonal 128x128 block on
                            # GPSIMD (idle in this phase) instead of a PE
                            # -1e30 matmul
                            nc.gpsimd.tensor_tensor(
                                es[:, lo:lo + 128], es[:, lo:lo + 128],
                                tri_s[:], OP.mult)
                        if jt == 0:
                            nc.vector.tensor_copy(racc[:], es[:])
                        else:
                            nc.vector.tensor_add(racc[:, lo:], racc[:, lo:],
                                                 es[:, lo:])
                        pend.append((jt, lo, es))
                        if prev is not None and prev["pend"]:
                            drain_one(prev)
                        elif len(pend) > 4:
                            drain_one(cur)
                        op_step()
                        # previous head finalizes mid-stream (once its AV
                        # drains are done) so its rowsum matmul never
                        # stalls the PE on the DVE racc chain
                        if (not fin_done and deferred[0] is not None
                                and jt >= min(2, njt - 1)
                                and not (prev is not None and prev["pend"])):
                            finalize_head(deferred[0])
                            deferred[0] = None
                            fin_done = True
                    if h == HPC - 1:
                        # last head of the chunk: flush everything
                        while (prev is not None and prev["pend"]) or pend:
                            if prev is not None and prev["pend"]:
                                drain_one(prev)
                            else:
                                drain_one(cur)
                            op_step()
                        prev = None
                    else:
                        while prev is not None and prev["pend"]:
                            drain_one(prev)
                            op_step()
                        prev = cur
                    if deferred[0] is not None:
                        finalize_head(deferred[0])
                        deferred[0] = None
                    deferred[0] = (ic, h, po, racc)
                if deferred[0] is not None:
                    finalize_head(deferred[0])
                    deferred[0] = None
                op_step(force=True)
            # tail: out projection of the last i-chunk, y spread on 4 rings
            op_begin(NCH - 1, 1, dmaq=[nc.sync, nc.gpsimd, nc.scalar])
            op_step(force=True)

    nc.compile()
    _NC_CACHE["nc"] = nc
    return nc


def _host_prep(rotary_pos_emb, w_rms, w_qkv, w_out):
    bf = ml_dtypes.bfloat16
    cos_t = np.ascontiguousarray(np.cos(rotary_pos_emb).T).astype(bf)
    # rotate_half via partition pair-swap: t2[2i] = raw[2i+1]*(-sin[2i]),
    # t2[2i+1] = raw[2i]*(+sin[2i+1]) -- fold the signs into the sin table
    sgn = np.where(np.arange(DH) % 2 == 0, -1.0, 1.0).astype(np.float32)
    sin_t = np.ascontiguousarray(
        (np.sin(rotary_pos_emb) * sgn[None, :]).T).astype(bf)
    ident = np.eye(128, dtype=bf)
    onesm = np.ones((128, 128), dtype=bf)
    # tri01[jj, ii] = 1 where ii >= jj (valid causal region of the
    # diagonal S^T block: partition=j, free=i)
    tri01 = (np.arange(128)[None, :] >= np.arange(128)[:, None])
    tri01 = np.ascontiguousarray(tri01).astype(bf)

    Ws = (w_qkv * w_rms[None, :]).astype(np.float32)  # fold RMSNorm weight
    per_core = []
    for g in range(2):
        rq = Ws[g * 1024:(g + 1) * 1024]              # q rows, heads 8g..
        rk = Ws[D + g * 1024:D + (g + 1) * 1024]      # k rows
        rv = Ws[2 * D + g * 1024:2 * D + (g + 1) * 1024]
        wqk_g = np.concatenate([rq, rk], 0).T.astype(bf)   # [D, 2048]
        wv_g = rv.T.astype(bf)                             # [D, 1024]
        wout_g = w_out[:, g * 1024:(g + 1) * 1024].T.astype(bf)  # [EV, D]
        # repack so device DMAs are contiguous per partition:
        # wqk [D=(dt p), e] -> [p, et, dt, 128]
        wqk_pk = np.ascontiguousarray(
            wqk_g.reshape(16, 128, 16, 128).transpose(1, 2, 0, 3))
        # wv [D=(dt p), e] -> [p, ech, dt, 512]
        wv_pk = np.ascontiguousarray(
            wv_g.reshape(16, 128, 2, 512).transpose(1, 2, 0, 3))
        # wout [EV=(et p), d] -> [p, et, d]
        wout_pk = np.ascontiguousarray(
            wout_g.reshape(8, 128, 2048).transpose(1, 0, 2))
        per_core.append(dict(wqk=wqk_pk, wv=wv_pk, wout=wout_pk,
                             cos_t=cos_t, sin_t=sin_t, ident=ident,
                             onesm=onesm, tri01=tri01))
    return per_core


def kernel(x, rotary_pos_emb, w_rms, w_qkv, w_out, _run=None):
    x = np.asarray(x, np.float32)
    rotary_pos_emb = np.asarray(rotary_pos_emb, np.float32)
    w_rms = np.asarray(w_rms, np.float32)
    w_qkv = np.asarray(w_qkv, np.float32)
    w_out = np.asarray(w_out, np.float32)

    nc = build_nc()
    groups = _host_prep(rotary_pos_emb, w_rms, w_qkv, w_out)
    in_maps = []
    for b in range(B):
        for g in range(2):
            m = dict(groups[g])
            m["x"] = np.ascontiguousarray(x[b])
            in_maps.append(m)
    if _run is None:
        res = run_bass_kernel_spmd(nc, in_maps, core_ids=list(range(8)))
        results = res.results
    else:
        results = _run(nc, in_maps)

    y = np.empty((B, N, D), np.float32)
    for b in range(B):
        y[b] = results[2 * b]["y"] + results[2 * b + 1]["y"]
    return y
